# revision 1
# baseline (speedup 1.0000x reference)
"""GQA self-attention block (q/k/v proj + causal softmax attention + o proj)
on 8 trn2 NeuronCores.

Sharding: batch (2) x query-head-groups (4) -> 8 cores. Core c handles
batch b=c//4 and heads [8g, 8g+8) where g=c%4 (kv heads [2g, 2g+2)).
Each core computes a partial output [T, D] = ctx_heads @ o_proj_cols.T;
the host sums the 4 partials per batch (all-reduce done host-side).

All matmuls run as float32r (fp32 rounded to 11-bit mantissa by the
producer, exact fp32 accumulation in PSUM) which streams at 1 cycle/row
for moving-dim >= 256.

Layout strategy (per core), everything d-major so no on-chip transposes of
x are needed (host passes x.T / weights pre-transposed):
  phase 1: qkvT[j, t] = wqkvT.T @ xT  (j = packed q|k|v output dims)
  phase 2: per head pair, S.T[j_keys, i_queries] = kT.T @ qT tiles
           (K=64 row-tiled at partitions 0/64 -> both heads concurrent),
           E = exp((S + mask)/8) on ACT, ctxT[e, i] accumulated as
           v_plus.T @ E with an appended ones column giving the softmax
           denominator in row 64; normalize with DVE reciprocal +
           PE broadcast + DVE multiply.
  phase 3: out[t, r] = ctxT.T @ o_projT, PSUM DMA'd straight to DRAM.
"""

import os
import sys

sys.path.insert(0, "/opt/trn_rl_repo")

import numpy as np

import concourse.bass as bass
import concourse.tile as tile
from concourse import bacc, mybir
from concourse.bass_utils import run_bass_kernel_spmd

F32 = mybir.dt.float32
F32R = mybir.dt.float32r
EXP = mybir.ActivationFunctionType.Exp

B, T, D = 2, 2048, 2048
HQ, HK = 32, 8
DH = D // HQ              # 64 head dim
N_CORES = 8
GROUPS = 4                # head groups per batch
QCOLS = D // GROUPS       # 512 q cols per core
KCOLS = (D // 4) // GROUPS  # 128 k cols per core (2 kv heads)
WCOLS = QCOLS + 2 * KCOLS   # 768
TB = 256                  # phase-1 token block
NTB = T // TB             # 8
KT = D // 128             # 16 contraction tiles
NEG = -480.0              # additive mask pre-scale (-60 after 1/8 scale)

_cache = {}


def _build():
    nc = bacc.Bacc("TRN2", target_bir_lowering=False, debug=False)

    xT_d = nc.declare_dram_parameter("xT", [D, T], F32R, isOutput=False)
    wqkv_d = nc.declare_dram_parameter("wqkv", [D, WCOLS], F32R, isOutput=False)
    oproj_d = nc.declare_dram_parameter("oproj", [QCOLS, D], F32R, isOutput=False)
    masks_d = nc.declare_dram_parameter("masks", [2, 128, 128], mybir.dt.bfloat16, isOutput=False)
    ident_d = nc.declare_dram_parameter("ident", [128, 64], F32R, isOutput=False)
    ones_d = nc.declare_dram_parameter("ones", [128, 16 * 65], F32R, isOutput=False)
    out_d = nc.declare_dram_parameter("out", [T, D], F32, isOutput=True)
    rcscr_d = nc.dram_tensor("rcscratch", [16, 1024], F32)

    with tile.TileContext(nc) as tc:
        with (
            tc.tile_pool(name="pers", bufs=1) as pers,
            tc.tile_pool(name="xt", bufs=24) as xtp,
            tc.tile_pool(name="work", bufs=2) as work,
            tc.tile_pool(name="psum", bufs=1, space="PSUM") as psum,
        ):
            # ---- constants / weights ----
            wqkv_sb = pers.tile([128, KT, WCOLS], F32R, tag="wslot")
            wq_r = wqkv_d[:].rearrange("(k p) c -> p k c", p=128)
            xt0 = []
            for k in range(KT):
                nc.sync.dma_start(
                    wqkv_sb[:, k, 0:128], wq_r[:, k, 0:128]
                )
                xt = xtp.tile([128, TB], F32R, tag="xt", bufs=24, name="xt0")
                nc.sync.dma_start(xt, xT_d[128 * k : 128 * k + 128, 0:TB])
                xt0.append(xt)
            for m in range(1, 6):
                csl = slice(128 * m, 128 * m + 128)
                for k in range(KT):
                    nc.sync.dma_start(wqkv_sb[:, k, csl], wq_r[:, k, csl])
            masks_sb = pers.tile([128, 2, 128], mybir.dt.bfloat16, tag="masks")
            ident_sb = pers.tile([128, 64], F32R, tag="ident")

            qt = [pers.tile([128, T], F32R, tag=f"qt{m}", name=f"qt{m}") for m in range(4)]
            kp = [pers.tile([128, T], F32R, tag=f"kp{k}", name=f"kp{k}") for k in range(2)]
            vT = pers.tile([128, T], F32R, tag="ctx0")
            vs = [pers.tile([128, 16 * 65], F32R, tag=f"vs{k}", name=f"vs{k}") for k in range(2)]
            ctx = [pers.tile([128, T], F32R, tag=f"ctx{m}", name=f"ctx{m}") for m in range(4)]

            nc.sync.dma_start(vs[0], ones_d[:])
            nc.sync.dma_start(vs[1], ones_d[:])

            # ---- phase 1: qkvT = wqkvT.T @ xT ----
            for tb in range(NTB):
                ts = slice(tb * TB, tb * TB + TB)
                if tb == 0:
                    xts = xt0
                else:
                    xts = []
                    for k in range(KT):
                        xt = xtp.tile([128, TB], F32R, tag="xt", bufs=24,
                                      name="xt")
                        nc.sync.dma_start(xt, xT_d[128 * k : 128 * k + 128, ts])
                        xts.append(xt)
                for m in range(6):
                    ps = psum.tile([128, TB], F32, tag="s2", bufs=4)
                    for k in range(KT):
                        nc.tensor.matmul(
                            ps,
                            wqkv_sb[:, k, 128 * m : 128 * m + 128],
                            xts[k],
                            start=(k == 0),
                            stop=(k == KT - 1),
                        )
                    if m < 4:
                        nc.scalar.copy(qt[m][:, ts], ps)
                    elif m == 4:
                        nc.scalar.copy(kp[0][0:64, ts], ps[0:64, :])
                        nc.scalar.copy(kp[1][64:128, ts], ps[64:128, :])
                    else:
                        nc.scalar.copy(vT[:, ts], ps)

            # constants needed from phase 1b on (emitted late so their DMAs
            # don't delay the phase-1 input stream)
            nc.sync.dma_start(
                masks_sb, masks_d[:].rearrange("o p f -> p o f")
            )
            nc.sync.dma_start(ident_sb, ident_d[:])

            # kT duplicates at the other partition half (SBUF->SBUF DMA
            # handles the partition shift)
            nc.sync.dma_start(kp[0][64:128, :], kp[0][0:64, :])
            nc.sync.dma_start(kp[1][0:64, :], kp[1][64:128, :])

            # ---- phase 1b: v = vT.T per 128-chunk, with ones column ----
            for kv in range(2):
                rows = slice(64 * kv, 64 * kv + 64)
                for c in range(16):
                    tp = psum.tile([128, 64], F32R, tag="s2", bufs=4)
                    nc.tensor.transpose(
                        tp,
                        vT[rows, 128 * c : 128 * c + 128],
                        ident_sb[rows, :],
                        tile_position=(64 * kv, 0),
                    )
                    nc.vector.tensor_copy(
                        vs[kv][:, 65 * c : 65 * c + 64], tp
                    )

            # ---- phase 2: attention per head pair ----
            # Per (head-pair m, query block a of 512): S.T pairs row-tiled at
            # partitions 0/64, exp on ACT, ctxT accumulated in PSUM with a
            # ones column giving the softmax denominator in row 64. Diagonal
            # 128-key chunks are trimmed to their valid query range with a
            # small [128,128] triangle mask. Normalization runs two blocks
            # deferred, on SBUF copies, so nothing ever waits on it.
            pending_norm = []
            for m in range(4):
                kv = m // 2
                for a in range(4):
                    nj = 4 * (a + 1)
                    isl = slice(512 * a, 512 * a + 512)
                    ctxAB = psum.tile([65, 1024], F32, tag="s2", bufs=4)
                    pend = []
                    for jc in range(nj):
                        if jc == 2 and len(pending_norm) >= 2:
                            pending_norm.pop(0)()
                        o = jc - 4 * a
                        lo = (0, 128, 256, 256)[o] if o >= 0 else 0
                        n = 512 - lo
                        jsl = slice(128 * jc, 128 * jc + 128)
                        S = psum.tile([128, 1024], F32, tag="s2", bufs=4)
                        for h2 in range(2):
                            nc.tensor.matmul(
                                S[:, 512 * h2 + lo : 512 * h2 + 512],
                                kp[kv][64 * h2 : 64 * h2 + 64, jsl],
                                qt[m][64 * h2 : 64 * h2 + 64,
                                      512 * a + lo : 512 * a + 512],
                                start=True,
                                stop=True,
                                tile_position=(64 * h2, 0),
                            )
                        if o >= 0:
                            tri = 128 * o  # triangle start col
                            for h2 in range(2):
                                base = 512 * h2
                                nc.vector.tensor_add(
                                    S[:, base + tri : base + tri + 128],
                                    S[:, base + tri : base + tri + 128],
                                    masks_sb[:, 0, :],
                                )
                                if o == 3:
                                    nc.vector.tensor_add(
                                        S[:, base + 256 : base + 384],
                                        S[:, base + 256 : base + 384],
                                        masks_sb[:, 1, :],
                                    )
                        E = work.tile([128, 1024], F32R, tag="E", bufs=4)
                        if lo == 0:
                            nc.scalar.activation(E, S, EXP, scale=0.125)
                        else:
                            for h2 in range(2):
                                base = 512 * h2
                                nc.scalar.activation(
                                    E[:, base + lo : base + 512],
                                    S[:, base + lo : base + 512],
                                    EXP,
                                    scale=0.125,
                                )
                        pend.append((E, jc, lo))
                        if len(pend) > 2:
                            pE, pjc, plo = pend.pop(0)
                            for h2 in range(2):
                                base = 512 * h2
                                nc.tensor.matmul(
                                    ctxAB[:, base + plo : base + 512],
                                    vs[kv][:, 65 * pjc : 65 * pjc + 65],
                                    pE[:, base + plo : base + 512],
                                    start=(pjc == 0),
                                    stop=False,
                                )
                    while pend:
                        pE, pjc, plo = pend.pop(0)
                        for h2 in range(2):
                            base = 512 * h2
                            nc.tensor.matmul(
                                ctxAB[:, base + plo : base + 512],
                                vs[kv][:, 65 * pjc : 65 * pjc + 65],
                                pE[:, base + plo : base + 512],
                                start=(pjc == 0),
                                stop=(pjc == nj - 1),
                            )

                    # one fast copy frees the ctx PSUM slot; the rest of the
                    # normalize chain runs two blocks later on the SBUF copy
                    cu = work.tile([65, 1024], F32, tag="cu", bufs=3,
                                   name="cu")
                    nc.vector.tensor_copy(cu, ctxAB)

                    def _normalize(cu=cu, m=m, a=a, isl=isl):
                        den128 = work.tile([128, 8], F32, tag="d128", bufs=2,
                                           name="den128")
                        nc.sync.dma_start(den128, cu[64:65, :])
                        rcp = work.tile([128, 8], F32, tag="rcp", bufs=2,
                                        name="rcp")
                        nc.vector.reciprocal(rcp, den128)
                        ma = m * 4 + a
                        nc.sync.dma_start(rcscr_d[ma : ma + 1, :], rcp)
                        bcs = work.tile([64, 1024], F32, tag="bcs", bufs=2,
                                        name="bcs")
                        nc.sync.dma_start(
                            bcs, rcscr_d[ma : ma + 1, :].partition_broadcast(64)
                        )
                        nc.vector.tensor_mul(
                            ctx[m][0:64, isl], cu[0:64, 0:512], bcs[:, 0:512]
                        )
                        tmpB = work.tile([64, 512], F32R, tag="tb", bufs=2)
                        nc.vector.tensor_mul(
                            tmpB, cu[0:64, 512:1024], bcs[:, 512:1024]
                        )
                        nc.sync.dma_start(ctx[m][64:128, isl], tmpB)

                    pending_norm.append(_normalize)

            while pending_norm:
                pending_norm.pop(0)()

            # ---- phase 3: out = ctxT.T @ o_projT ----
            oproj_sb = pers.tile([128, 4, D], F32R, tag="wslot")
            nc.sync.dma_start(
                oproj_sb, oproj_d[:].rearrange("(m p) c -> p m c", p=128)
            )
            for t in range(16):
                tsl = slice(128 * t, 128 * t + 128)
                for rp in range(2):
                    ps = psum.tile([128, 1024], F32, tag="s2", bufs=4)
                    for m in range(4):
                        for half in range(2):
                            r = 2 * rp + half
                            nc.tensor.matmul(
                                ps[:, 512 * half : 512 * half + 512],
                                ctx[m][:, tsl],
                                oproj_sb[:, m, 512 * r : 512 * r + 512],
                                start=(m == 0),
                                stop=(m == 3),
                            )
                    ostage = work.tile([128, 1024], F32, tag="E", bufs=4,
                                       name="ostage")
                    nc.scalar.copy(ostage, ps)
                    nc.sync.dma_start(
                        out_d[tsl, 1024 * rp : 1024 * rp + 1024], ostage
                    )

    nc.compile()
    return nc


def _host_inputs(x, q_proj, k_proj, v_proj, o_proj):
    """Per-core input dicts (numpy, float32)."""
    masks = np.zeros((2, 128, 128), dtype=np.float32)
    jj = np.arange(128)[:, None]
    ii = np.arange(128)[None, :]
    masks[0] = np.where(jj <= ii, 0.0, NEG)
    masks[1] = NEG
    import ml_dtypes
    masks = masks.astype(ml_dtypes.bfloat16)
    ident = np.zeros((128, 64), dtype=np.float32)
    ident[np.arange(64), np.arange(64)] = 1.0
    ident[np.arange(64) + 64, np.arange(64)] = 1.0

    xT = [np.ascontiguousarray(x[b].T) for b in range(B)]
    in_maps = []
    for c in range(N_CORES):
        b, g = divmod(c, GROUPS)
        wqkv = np.concatenate(
            [
                q_proj[QCOLS * g : QCOLS * g + QCOLS].T,
                k_proj[KCOLS * g : KCOLS * g + KCOLS].T,
                v_proj[KCOLS * g : KCOLS * g + KCOLS].T,
            ],
            axis=1,
        )
        in_maps.append(
            {
                "xT": xT[b],
                "wqkv": np.ascontiguousarray(wqkv),
                "oproj": np.ascontiguousarray(
                    o_proj[:, QCOLS * g : QCOLS * g + QCOLS].T
                ),
                "masks": masks,
                "ident": ident,
                "ones": np.ones((128, 16 * 65), dtype=np.float32),
            }
        )
    return in_maps


def run(x, q_proj, k_proj, v_proj, o_proj, trace=False):
    """Run on hardware; returns (output [B,T,D] f32, BassKernelResults)."""
    if "nc" not in _cache:
        _cache["nc"] = _build()
    nc = _cache["nc"]
    in_maps = _host_inputs(x, q_proj, k_proj, v_proj, o_proj)
    res = run_bass_kernel_spmd(
        nc, in_maps, core_ids=list(range(N_CORES)), trace=trace
    )
    parts = [res.results[c]["out"] for c in range(N_CORES)]
    out = np.empty((B, T, D), dtype=np.float32)
    for b in range(B):
        acc = parts[4 * b].astype(np.float64)
        for g in range(1, GROUPS):
            acc += parts[4 * b + g]
        out[b] = acc.astype(np.float32)
    return out, res


def kernel(x, q_proj, k_proj, v_proj, o_proj, hq=None, hk=None, **_unused):
    x = np.asarray(x, dtype=np.float32)
    q_proj = np.asarray(q_proj, dtype=np.float32)
    k_proj = np.asarray(k_proj, dtype=np.float32)
    v_proj = np.asarray(v_proj, dtype=np.float32)
    o_proj = np.asarray(o_proj, dtype=np.float32)
    assert x.shape == (B, T, D), x.shape
    trace = bool(os.environ.get("KERNEL_TRACE"))
    out, _ = run(x, q_proj, k_proj, v_proj, o_proj, trace=trace)
    return out



# revision 3
# speedup vs baseline: 1.0790x; 1.0790x over previous
"""GQA self-attention block (q/k/v proj + causal softmax attention + o proj)
on 8 trn2 NeuronCores.

Sharding: batch (2) x query-head-groups (4) -> 8 cores. Core c handles
batch b=c//4 and heads [8g, 8g+8) where g=c%4 (kv heads [2g, 2g+2)).
Each core computes a partial output [T, D] = ctx_heads @ o_proj_cols.T;
the host sums the 4 partials per batch (all-reduce done host-side).

All matmuls run in bf16 (fast weight load via FWL + 1 col/cycle streaming;
fp32 dtypes are excluded from FWL and stall on LDWEIGHTS), accumulation in
fp32 PSUM.

Layout strategy (per core), everything d-major so no on-chip transposes of
x are needed (host passes x.T / weights pre-transposed):
  phase 1: qkvT[j, t] = wqkvT.T @ xT  (j = packed q|k|v output dims)
  phase 2: per head pair, S.T[j_keys, i_queries] = kT.T @ qT tiles
           (K=64 row-tiled at partitions 0/64 -> both heads concurrent),
           E = exp((S + mask)/8) on ACT, ctxT[e, i] accumulated as
           v_plus.T @ E with an appended ones column giving the softmax
           denominator in row 64; normalize with DVE reciprocal +
           PE broadcast + DVE multiply.
  phase 3: out[t, r] = ctxT.T @ o_projT, staged through SBUF to DRAM.
"""

import os
import sys

sys.path.insert(0, "/opt/trn_rl_repo")

import numpy as np

import concourse.bass as bass
import concourse.tile as tile
from concourse import bacc, mybir
from concourse.bass_utils import run_bass_kernel_spmd

F32 = mybir.dt.float32
BF16 = mybir.dt.bfloat16
EXP = mybir.ActivationFunctionType.Exp

B, T, D = 2, 2048, 2048
HQ, HK = 32, 8
DH = D // HQ              # 64 head dim
N_CORES = 8
GROUPS = 4                # head groups per batch
QCOLS = D // GROUPS       # 512 q cols per core
KCOLS = (D // 4) // GROUPS  # 128 k cols per core (2 kv heads)
WCOLS = QCOLS + 2 * KCOLS   # 768
TB = 512                  # phase-1 token block
NTB = T // TB             # 4
KT = D // 128             # 16 contraction tiles
NEG = -480.0              # additive mask pre-scale (-60 after 1/8 scale)

_cache = {}


def _build():
    nc = bacc.Bacc("TRN2", target_bir_lowering=False, debug=False)

    xT_d = nc.declare_dram_parameter("xT", [D, T], BF16, isOutput=False)
    wqkv_d = nc.declare_dram_parameter("wqkv", [D, WCOLS], BF16, isOutput=False)
    oproj_d = nc.declare_dram_parameter("oproj", [QCOLS, D], BF16, isOutput=False)
    masks_d = nc.declare_dram_parameter("masks", [2, 128, 128], BF16, isOutput=False)
    ident_d = nc.declare_dram_parameter("ident", [128, 64], BF16, isOutput=False)
    ones_d = nc.declare_dram_parameter("ones", [128, 16 * 65], BF16, isOutput=False)
    out_d = nc.declare_dram_parameter("out", [T, D], F32, isOutput=True)
    rcscr_d = nc.dram_tensor("rcscratch", [16, 1024], F32)

    with tile.TileContext(nc) as tc:
        with (
            tc.tile_pool(name="pers", bufs=1) as pers,
            tc.tile_pool(name="xt", bufs=24) as xtp,
            tc.tile_pool(name="work", bufs=2) as work,
            tc.tile_pool(name="psum", bufs=1, space="PSUM") as psum,
        ):
            # ---- constants / weights ----
            wqkv_sb = pers.tile([128, KT, WCOLS], BF16, tag="wslot")
            wq_r = wqkv_d[:].rearrange("(k p) c -> p k c", p=128)
            xt0 = []
            for k in range(KT):
                nc.sync.dma_start(
                    wqkv_sb[:, k, 0:128], wq_r[:, k, 0:128]
                )
                xt = xtp.tile([128, TB], BF16, tag="xt", bufs=32, name="xt0")
                nc.sync.dma_start(xt, xT_d[128 * k : 128 * k + 128, 0:TB])
                xt0.append(xt)
            for m in range(1, 6):
                csl = slice(128 * m, 128 * m + 128)
                for k in range(KT):
                    nc.sync.dma_start(wqkv_sb[:, k, csl], wq_r[:, k, csl])
            masks_sb = pers.tile([128, 2, 128], BF16, tag="masks")
            ident_sb = pers.tile([128, 64], BF16, tag="ident")

            qt = [pers.tile([128, T], BF16, tag=f"qt{m}", name=f"qt{m}") for m in range(4)]
            kp = [pers.tile([128, T], BF16, tag=f"kp{k}", name=f"kp{k}") for k in range(2)]
            vT = pers.tile([128, T], BF16, tag="ctx0")
            vs = [pers.tile([128, 16 * 65], BF16, tag=f"vs{k}", name=f"vs{k}") for k in range(2)]
            ctx = [pers.tile([128, T], BF16, tag=f"ctx{m}", name=f"ctx{m}") for m in range(4)]

            nc.sync.dma_start(vs[0], ones_d[:])
            nc.sync.dma_start(vs[1], ones_d[:])

            # ---- phase 1: qkvT = wqkvT.T @ xT ----
            for tb in range(NTB):
                ts = slice(tb * TB, tb * TB + TB)
                if tb == 0:
                    xts = xt0
                else:
                    xts = []
                    for k in range(KT):
                        xt = xtp.tile([128, TB], BF16, tag="xt", bufs=32,
                                      name="xt")
                        nc.sync.dma_start(xt, xT_d[128 * k : 128 * k + 128, ts])
                        xts.append(xt)
                for m in range(6):
                    ps = psum.tile([128, TB], F32, tag="s2", bufs=4)
                    for k in range(KT):
                        nc.tensor.matmul(
                            ps,
                            wqkv_sb[:, k, 128 * m : 128 * m + 128],
                            xts[k],
                            start=(k == 0),
                            stop=(k == KT - 1),
                        )
                    if m < 4:
                        nc.scalar.copy(qt[m][:, ts], ps)
                    elif m == 4:
                        nc.scalar.copy(kp[0][0:64, ts], ps[0:64, :])
                        nc.scalar.copy(kp[1][64:128, ts], ps[64:128, :])
                    else:
                        nc.scalar.copy(vT[:, ts], ps)

            # constants needed from phase 1b on (emitted late so their DMAs
            # don't delay the phase-1 input stream)
            nc.sync.dma_start(
                masks_sb, masks_d[:].rearrange("o p f -> p o f")
            )
            nc.sync.dma_start(ident_sb, ident_d[:])

            # kT duplicates at the other partition half (SBUF->SBUF DMA
            # handles the partition shift)
            nc.sync.dma_start(kp[0][64:128, :], kp[0][0:64, :])
            nc.sync.dma_start(kp[1][0:64, :], kp[1][64:128, :])

            # ---- phase 1b: v = vT.T per 128-chunk, with ones column ----
            for kv in range(2):
                rows = slice(64 * kv, 64 * kv + 64)
                for c in range(16):
                    tp = psum.tile([128, 64], BF16, tag="s2", bufs=4)
                    nc.tensor.transpose(
                        tp,
                        vT[rows, 128 * c : 128 * c + 128],
                        ident_sb[rows, :],
                        tile_position=(64 * kv, 0),
                    )
                    nc.vector.tensor_copy(
                        vs[kv][:, 65 * c : 65 * c + 64], tp
                    )

            # ---- phase 2: attention per head pair ----
            # Per (head-pair m, query block a of 512): S.T pairs row-tiled at
            # partitions 0/64, exp on ACT, ctxT accumulated in PSUM with a
            # ones column giving the softmax denominator in row 64. Diagonal
            # 128-key chunks are trimmed to their valid query range with a
            # small [128,128] triangle mask. Normalization runs two blocks
            # deferred, on SBUF copies, so nothing ever waits on it.
            pending_norm = []
            for m in range(4):
                kv = m // 2
                for a in range(4):
                    nj = 4 * (a + 1)
                    isl = slice(512 * a, 512 * a + 512)
                    ctxAB = psum.tile([65, 1024], F32, tag="s2", bufs=4)
                    pend = []
                    for jc in range(nj):
                        if jc == 2 and len(pending_norm) >= 2:
                            pending_norm.pop(0)()
                        o = jc - 4 * a
                        lo = (0, 128, 256, 256)[o] if o >= 0 else 0
                        n = 512 - lo
                        jsl = slice(128 * jc, 128 * jc + 128)
                        S = psum.tile([128, 1024], F32, tag="s2", bufs=4)
                        for h2 in range(2):
                            nc.tensor.matmul(
                                S[:, 512 * h2 + lo : 512 * h2 + 512],
                                kp[kv][64 * h2 : 64 * h2 + 64, jsl],
                                qt[m][64 * h2 : 64 * h2 + 64,
                                      512 * a + lo : 512 * a + 512],
                                start=True,
                                stop=True,
                                tile_position=(64 * h2, 0),
                            )
                        if o >= 0:
                            tri = 128 * o  # triangle start col
                            for h2 in range(2):
                                base = 512 * h2
                                nc.vector.tensor_add(
                                    S[:, base + tri : base + tri + 128],
                                    S[:, base + tri : base + tri + 128],
                                    masks_sb[:, 0, :],
                                )
                                if o == 3:
                                    nc.vector.tensor_add(
                                        S[:, base + 256 : base + 384],
                                        S[:, base + 256 : base + 384],
                                        masks_sb[:, 1, :],
                                    )
                        E = work.tile([128, 1024], BF16, tag="E", bufs=4)
                        if lo == 0:
                            nc.scalar.activation(E, S, EXP, scale=0.125)
                        else:
                            for h2 in range(2):
                                base = 512 * h2
                                nc.scalar.activation(
                                    E[:, base + lo : base + 512],
                                    S[:, base + lo : base + 512],
                                    EXP,
                                    scale=0.125,
                                )
                        pend.append((E, jc, lo))
                        if len(pend) > 2:
                            pE, pjc, plo = pend.pop(0)
                            for h2 in range(2):
                                base = 512 * h2
                                nc.tensor.matmul(
                                    ctxAB[:, base + plo : base + 512],
                                    vs[kv][:, 65 * pjc : 65 * pjc + 65],
                                    pE[:, base + plo : base + 512],
                                    start=(pjc == 0),
                                    stop=False,
                                )
                    while pend:
                        pE, pjc, plo = pend.pop(0)
                        for h2 in range(2):
                            base = 512 * h2
                            nc.tensor.matmul(
                                ctxAB[:, base + plo : base + 512],
                                vs[kv][:, 65 * pjc : 65 * pjc + 65],
                                pE[:, base + plo : base + 512],
                                start=(pjc == 0),
                                stop=(pjc == nj - 1),
                            )

                    # one fast copy frees the ctx PSUM slot; the rest of the
                    # normalize chain runs two blocks later on the SBUF copy
                    cu = work.tile([65, 1024], F32, tag="cu", bufs=3,
                                   name="cu")
                    nc.vector.tensor_copy(cu, ctxAB)

                    def _normalize(cu=cu, m=m, a=a, isl=isl):
                        den128 = work.tile([128, 8], F32, tag="d128", bufs=2,
                                           name="den128")
                        nc.sync.dma_start(den128, cu[64:65, :])
                        rcp = work.tile([128, 8], F32, tag="rcp", bufs=2,
                                        name="rcp")
                        nc.vector.reciprocal(rcp, den128)
                        ma = m * 4 + a
                        nc.sync.dma_start(rcscr_d[ma : ma + 1, :], rcp)
                        bcs = work.tile([64, 1024], F32, tag="bcs", bufs=2,
                                        name="bcs")
                        nc.sync.dma_start(
                            bcs, rcscr_d[ma : ma + 1, :].partition_broadcast(64)
                        )
                        nc.vector.tensor_mul(
                            ctx[m][0:64, isl], cu[0:64, 0:512], bcs[:, 0:512]
                        )
                        tmpB = work.tile([64, 512], BF16, tag="tb", bufs=2)
                        nc.vector.tensor_mul(
                            tmpB, cu[0:64, 512:1024], bcs[:, 512:1024]
                        )
                        nc.sync.dma_start(ctx[m][64:128, isl], tmpB)

                    pending_norm.append(_normalize)

            while pending_norm:
                pending_norm.pop(0)()

            # ---- phase 3: out = ctxT.T @ o_projT ----
            oproj_sb = pers.tile([128, 4, D], BF16, tag="wslot")
            nc.sync.dma_start(
                oproj_sb, oproj_d[:].rearrange("(m p) c -> p m c", p=128)
            )
            for t in range(16):
                tsl = slice(128 * t, 128 * t + 128)
                for rp in range(2):
                    ps = psum.tile([128, 1024], F32, tag="s2", bufs=4)
                    for m in range(4):
                        for half in range(2):
                            r = 2 * rp + half
                            nc.tensor.matmul(
                                ps[:, 512 * half : 512 * half + 512],
                                ctx[m][:, tsl],
                                oproj_sb[:, m, 512 * r : 512 * r + 512],
                                start=(m == 0),
                                stop=(m == 3),
                            )
                    ostage = work.tile([128, 1024], F32, tag="ost", bufs=4,
                                       name="ostage")
                    nc.scalar.copy(ostage, ps)
                    nc.sync.dma_start(
                        out_d[tsl, 1024 * rp : 1024 * rp + 1024], ostage
                    )

    nc.compile()
    return nc


def _host_inputs(x, q_proj, k_proj, v_proj, o_proj):
    """Per-core input dicts (numpy, bf16 on-device dtypes)."""
    import ml_dtypes

    bf16 = ml_dtypes.bfloat16
    masks = np.zeros((2, 128, 128), dtype=np.float32)
    jj = np.arange(128)[:, None]
    ii = np.arange(128)[None, :]
    masks[0] = np.where(jj <= ii, 0.0, NEG)
    masks[1] = NEG
    masks = masks.astype(bf16)
    ident = np.zeros((128, 64), dtype=np.float32)
    ident[np.arange(64), np.arange(64)] = 1.0
    ident[np.arange(64) + 64, np.arange(64)] = 1.0
    ident = ident.astype(bf16)

    xT = [np.ascontiguousarray(x[b].T).astype(bf16) for b in range(B)]
    in_maps = []
    for c in range(N_CORES):
        b, g = divmod(c, GROUPS)
        wqkv = np.concatenate(
            [
                q_proj[QCOLS * g : QCOLS * g + QCOLS].T,
                k_proj[KCOLS * g : KCOLS * g + KCOLS].T,
                v_proj[KCOLS * g : KCOLS * g + KCOLS].T,
            ],
            axis=1,
        )
        in_maps.append(
            {
                "xT": xT[b],
                "wqkv": np.ascontiguousarray(wqkv).astype(bf16),
                "oproj": np.ascontiguousarray(
                    o_proj[:, QCOLS * g : QCOLS * g + QCOLS].T
                ).astype(bf16),
                "masks": masks,
                "ident": ident,
                "ones": np.ones((128, 16 * 65), dtype=bf16),
            }
        )
    return in_maps


def run(x, q_proj, k_proj, v_proj, o_proj, trace=False):
    """Run on hardware; returns (output [B,T,D] f32, BassKernelResults)."""
    if "nc" not in _cache:
        _cache["nc"] = _build()
    nc = _cache["nc"]
    in_maps = _host_inputs(x, q_proj, k_proj, v_proj, o_proj)
    res = run_bass_kernel_spmd(
        nc, in_maps, core_ids=list(range(N_CORES)), trace=trace
    )
    parts = [res.results[c]["out"] for c in range(N_CORES)]
    out = np.empty((B, T, D), dtype=np.float32)
    for b in range(B):
        acc = parts[4 * b].astype(np.float64)
        for g in range(1, GROUPS):
            acc += parts[4 * b + g]
        out[b] = acc.astype(np.float32)
    return out, res


def kernel(x, q_proj, k_proj, v_proj, o_proj, hq=None, hk=None, **_unused):
    x = np.asarray(x, dtype=np.float32)
    q_proj = np.asarray(q_proj, dtype=np.float32)
    k_proj = np.asarray(k_proj, dtype=np.float32)
    v_proj = np.asarray(v_proj, dtype=np.float32)
    o_proj = np.asarray(o_proj, dtype=np.float32)
    assert x.shape == (B, T, D), x.shape
    trace = bool(os.environ.get("KERNEL_TRACE"))
    out, _ = run(x, q_proj, k_proj, v_proj, o_proj, trace=trace)
    return out


# revision 4
# speedup vs baseline: 1.3385x; 1.2405x over previous
"""GQA self-attention block (q/k/v proj + causal softmax attention + o proj)
on 8 trn2 NeuronCores.

Sharding: batch (2) x query-head-groups (4) -> 8 cores. Core c handles
batch b=c//4 and heads [8g, 8g+8) where g=c%4 (kv heads [2g, 2g+2)).
Each core computes a partial output [T, D] = ctx_heads @ o_proj_cols.T;
the host sums the 4 partials per batch (all-reduce done host-side).

All matmuls bf16 (FWL weight loads + 1 col/cycle streams), fp32 PSUM.

Single software-pipelined schedule: query blocks a=0..3 outer, head pairs
m inner. Phase-1 token-block b=a+1 and phase-3 output tiles of block a-1
are interleaved into the attention chunk stream as PE filler so the tensor
engine never idles while ACT works through the exps.
"""

import os
import sys

sys.path.insert(0, "/opt/trn_rl_repo")

import numpy as np

import concourse.bass as bass
import concourse.tile as tile
from concourse import bacc, mybir
from concourse.bass_utils import run_bass_kernel_spmd

F32 = mybir.dt.float32
BF16 = mybir.dt.bfloat16
EXP = mybir.ActivationFunctionType.Exp

B, T, D = 2, 2048, 2048
HQ, HK = 32, 8
N_CORES = 8
GROUPS = 4
QCOLS = D // GROUPS         # 512 q cols per core
KCOLS = (D // 4) // GROUPS  # 128 k cols per core (2 kv heads)
WCOLS = QCOLS + 2 * KCOLS   # 768
TB = 512                    # phase-1 token block
NTB = T // TB               # 4
KT = D // 128               # 16 contraction tiles
NEG = -480.0                # additive mask pre-scale (-60 after 1/8 scale)

_cache = {}


def _build():
    nc = bacc.Bacc("TRN2", target_bir_lowering=False, debug=False)

    xT_d = nc.declare_dram_parameter("xT", [D, T], BF16, isOutput=False)
    wqkv_d = nc.declare_dram_parameter("wqkv", [D, WCOLS], BF16, isOutput=False)
    oproj_d = nc.declare_dram_parameter("oproj", [QCOLS, D], BF16, isOutput=False)
    masks_d = nc.declare_dram_parameter("masks", [128, 2, 128], BF16, isOutput=False)
    ident_d = nc.declare_dram_parameter("ident", [128, 64], BF16, isOutput=False)
    ones_d = nc.declare_dram_parameter("ones", [128, 16 * 65], BF16, isOutput=False)
    out_d = nc.declare_dram_parameter("out", [T, D], F32, isOutput=True)
    rcscr_d = nc.dram_tensor("rcscratch", [16, 1024], F32)

    with tile.TileContext(nc) as tc:
        with (
            tc.tile_pool(name="pers", bufs=1) as pers,
            tc.tile_pool(name="xt", bufs=32) as xtp,
            tc.tile_pool(name="work", bufs=2) as work,
            tc.tile_pool(name="psum", bufs=1, space="PSUM") as psum,
        ):
            # ---- persistent tiles ----
            wqkv_sb = pers.tile([128, KT, WCOLS], BF16, tag="wslot")
            oproj_sb = pers.tile([128, 4, D], BF16, tag="oslot")
            masks_sb = pers.tile([128, 2, 128], BF16, tag="masks")
            ident_sb = pers.tile([128, 64], BF16, tag="ident")
            qt = [pers.tile([128, T], BF16, tag=f"qt{m}", name=f"qt{m}")
                  for m in range(4)]
            kp = [pers.tile([128, T], BF16, tag=f"kp{k}", name=f"kp{k}")
                  for k in range(2)]
            vT = pers.tile([128, T], BF16, tag="vT")
            vs = [pers.tile([128, 16 * 65], BF16, tag=f"vs{k}", name=f"vs{k}")
                  for k in range(2)]
            ctx = [pers.tile([128, T], BF16, tag=f"ctx{m}", name=f"ctx{m}")
                   for m in range(4)]

            # ---- input DMAs for tb0 + constants ----
            wq_r = wqkv_d[:].rearrange("(k p) c -> p k c", p=128)
            xt0 = []
            for k in range(KT):
                nc.sync.dma_start(wqkv_sb[:, k, 0:128], wq_r[:, k, 0:128])
                xt = xtp.tile([128, TB], BF16, tag="xt", bufs=32, name="xt0")
                nc.sync.dma_start(xt, xT_d[128 * k : 128 * k + 128, 0:TB])
                xt0.append(xt)
            for m in range(1, 6):
                csl = slice(128 * m, 128 * m + 128)
                for k in range(KT):
                    nc.sync.dma_start(wqkv_sb[:, k, csl], wq_r[:, k, csl])
            nc.sync.dma_start(masks_sb, masks_d[:])
            nc.sync.dma_start(ident_sb, ident_d[:])
            nc.sync.dma_start(vs[0], ones_d[:])
            nc.sync.dma_start(vs[1], ones_d[:])
            nc.sync.dma_start(
                oproj_sb, oproj_d[:].rearrange("(m p) c -> p m c", p=128)
            )

            # ---- phase-1 emission helpers ----
            def p1_dma(tb):
                xts = []
                ts = slice(tb * TB, tb * TB + TB)
                for k in range(KT):
                    xt = xtp.tile([128, TB], BF16, tag="xt", bufs=32, name="xt")
                    nc.sync.dma_start(xt, xT_d[128 * k : 128 * k + 128, ts])
                    xts.append(xt)
                return xts

            def p1_group(tb, m, xts):
                ts = slice(tb * TB, tb * TB + TB)
                ps = psum.tile([128, TB], F32, tag="s2", bufs=4)
                for k in range(KT):
                    nc.tensor.matmul(
                        ps,
                        wqkv_sb[:, k, 128 * m : 128 * m + 128],
                        xts[k],
                        start=(k == 0),
                        stop=(k == KT - 1),
                    )
                if m < 4:
                    nc.scalar.copy(qt[m][:, ts], ps)
                elif m == 4:
                    nc.scalar.copy(kp[0][0:64, ts], ps[0:64, :])
                    nc.scalar.copy(kp[1][64:128, ts], ps[64:128, :])
                else:
                    nc.scalar.copy(vT[:, ts], ps)

            def p1_final(tb):
                # k duplicate at the other partition half + v transposes
                ts = slice(tb * TB, tb * TB + TB)
                nc.sync.dma_start(kp[0][64:128, ts], kp[0][0:64, ts])
                nc.sync.dma_start(kp[1][0:64, ts], kp[1][64:128, ts])
                for kv in range(2):
                    rows = slice(64 * kv, 64 * kv + 64)
                    for ci in range(4):
                        c = 4 * tb + ci
                        tp = psum.tile([128, 64], BF16, tag="s2", bufs=4)
                        nc.tensor.transpose(
                            tp,
                            vT[rows, 128 * c : 128 * c + 128],
                            ident_sb[rows, :],
                            tile_position=(64 * kv, 0),
                        )
                        nc.vector.tensor_copy(vs[kv][:, 65 * c : 65 * c + 64], tp)

            # ---- phase-3 emission helper (one [128-token, 1024-col] unit) ----
            def p3_unit(t, rp):
                tsl = slice(128 * t, 128 * t + 128)
                ps = psum.tile([128, 1024], F32, tag="s2", bufs=4)
                for m in range(4):
                    for half in range(2):
                        r = 2 * rp + half
                        nc.tensor.matmul(
                            ps[:, 512 * half : 512 * half + 512],
                            ctx[m][:, tsl],
                            oproj_sb[:, m, 512 * r : 512 * r + 512],
                            start=(m == 0),
                            stop=(m == 3),
                        )
                ostage = work.tile([128, 1024], F32, tag="ost", bufs=4,
                                   name="ostage")
                if rp == 0:
                    nc.vector.tensor_copy(ostage, ps)
                else:
                    nc.scalar.copy(ostage, ps)
                nc.sync.dma_start(
                    out_d[tsl, 1024 * rp : 1024 * rp + 1024], ostage
                )

            # ---- run phase 1 for tb0 ----
            for m in range(6):
                p1_group(0, m, xt0)
            p1_final(0)

            # ---- main pipeline: a outer, m inner ----
            pending_norm = []

            def emit_norm(cu, m, a):
                isl = slice(512 * a, 512 * a + 512)

                def _normalize(cu=cu, m=m, a=a, isl=isl):
                    den128 = work.tile([128, 8], F32, tag="d128", bufs=2,
                                       name="den128")
                    nc.sync.dma_start(den128, cu[64:65, :])
                    rcp = work.tile([128, 8], F32, tag="rcp", bufs=2,
                                    name="rcp")
                    nc.vector.reciprocal(rcp, den128)
                    ma = m * 4 + a
                    nc.sync.dma_start(rcscr_d[ma : ma + 1, :], rcp)
                    bcs = work.tile([64, 1024], F32, tag="bcs", bufs=2,
                                    name="bcs")
                    nc.sync.dma_start(
                        bcs, rcscr_d[ma : ma + 1, :].partition_broadcast(64)
                    )
                    nc.vector.tensor_mul(
                        ctx[m][0:64, isl], cu[0:64, 0:512], bcs[:, 0:512]
                    )
                    tmpB = work.tile([64, 512], BF16, tag="tb", bufs=2)
                    nc.vector.tensor_mul(
                        tmpB, cu[0:64, 512:1024], bcs[:, 512:1024]
                    )
                    nc.sync.dma_start(ctx[m][64:128, isl], tmpB)

                pending_norm.append(_normalize)

            for a in range(4):
                nj = 4 * (a + 1)
                isl = slice(512 * a, 512 * a + 512)

                # plan PE filler work for this a-block, keyed by head pair m
                fillers = {0: [], 1: [], 2: [], 3: []}
                if a + 1 < NTB:
                    tbn = a + 1
                    xts_n = p1_dma(tbn)
                    for mm in range(6):
                        fillers[mm // 3].append(
                            lambda tb=tbn, mm=mm, xx=xts_n: p1_group(tb, mm, xx)
                        )
                    fillers[1].append(lambda tb=tbn: p1_final(tb))
                if a >= 1:
                    ap = a - 1
                    for ui, (t, rp) in enumerate(
                        (4 * ap + ti, rp) for ti in range(4) for rp in range(2)
                    ):
                        fillers[2 + (ui // 4)].append(
                            lambda t=t, rp=rp: p3_unit(t, rp)
                        )

                for m in range(4):
                    kv = m // 2
                    ctxAB = psum.tile([65, 1024], F32, tag="s2", bufs=4)
                    pend = []
                    fl = fillers[m]
                    # spread fillers across the chunk loop
                    fpos = {}
                    if fl:
                        step = max(1, nj // len(fl))
                        for fi in range(len(fl)):
                            fpos.setdefault(min(nj - 1, fi * step + 1), []).append(fl[fi])
                    for jc in range(nj):
                        if jc == 2 and len(pending_norm) >= 2:
                            pending_norm.pop(0)()
                        o = jc - 4 * a
                        lo = 128 * o if o >= 0 else 0
                        jsl = slice(128 * jc, 128 * jc + 128)
                        S = psum.tile([128, 2, 512], F32, tag="s2", bufs=4)
                        for h2 in range(2):
                            nc.tensor.matmul(
                                S[:, h2, lo:512],
                                kp[kv][64 * h2 : 64 * h2 + 64, jsl],
                                qt[m][64 * h2 : 64 * h2 + 64,
                                      512 * a + lo : 512 * a + 512],
                                start=True,
                                stop=True,
                                tile_position=(64 * h2, 0),
                            )
                        if o >= 0:
                            # triangle mask on the diagonal 128-col block,
                            # both heads in one 2-range DVE add
                            nc.vector.tensor_add(
                                S[:, :, lo : lo + 128],
                                S[:, :, lo : lo + 128],
                                masks_sb[:, :, :],
                            )
                        E = work.tile([128, 2, 512], BF16, tag="E", bufs=4)
                        nc.scalar.activation(
                            E[:, :, lo:512], S[:, :, lo:512], EXP, scale=0.125
                        )
                        pend.append((E, jc, lo))
                        if len(pend) > 2:
                            pE, pjc, plo = pend.pop(0)
                            for h2 in range(2):
                                nc.tensor.matmul(
                                    ctxAB[:, 512 * h2 + plo : 512 * h2 + 512],
                                    vs[kv][:, 65 * pjc : 65 * pjc + 65],
                                    pE[:, h2, plo:512],
                                    start=(pjc == 0),
                                    stop=False,
                                )
                        for f in fpos.get(jc, ()):
                            f()
                    while pend:
                        pE, pjc, plo = pend.pop(0)
                        for h2 in range(2):
                            nc.tensor.matmul(
                                ctxAB[:, 512 * h2 + plo : 512 * h2 + 512],
                                vs[kv][:, 65 * pjc : 65 * pjc + 65],
                                pE[:, h2, plo:512],
                                start=(pjc == 0),
                                stop=(pjc == nj - 1),
                            )

                    # fast PSUM evacuation; the normalize chain runs deferred
                    cu = work.tile([65, 1024], F32, tag="cu", bufs=3, name="cu")
                    nc.vector.tensor_copy(cu, ctxAB)
                    emit_norm(cu, m, a)

            # ---- drain: last norms + last phase-3 block ----
            while pending_norm:
                pending_norm.pop(0)()
            for ti in range(4):
                for rp in range(2):
                    p3_unit(12 + ti, rp)

    nc.compile()
    return nc


def _host_inputs(x, q_proj, k_proj, v_proj, o_proj):
    """Per-core input dicts (numpy, bf16 on-device dtypes)."""
    import ml_dtypes

    bf16 = ml_dtypes.bfloat16
    masks = np.zeros((128, 2, 128), dtype=np.float32)
    jj = np.arange(128)[:, None]
    ii = np.arange(128)[None, :]
    tri = np.where(jj <= ii, 0.0, NEG)
    masks[:, 0, :] = tri
    masks[:, 1, :] = tri
    masks = masks.astype(bf16)
    ident = np.zeros((128, 64), dtype=np.float32)
    ident[np.arange(64), np.arange(64)] = 1.0
    ident[np.arange(64) + 64, np.arange(64)] = 1.0
    ident = ident.astype(bf16)

    xT = [np.ascontiguousarray(x[b].T).astype(bf16) for b in range(B)]
    in_maps = []
    for c in range(N_CORES):
        b, g = divmod(c, GROUPS)
        wqkv = np.concatenate(
            [
                q_proj[QCOLS * g : QCOLS * g + QCOLS].T,
                k_proj[KCOLS * g : KCOLS * g + KCOLS].T,
                v_proj[KCOLS * g : KCOLS * g + KCOLS].T,
            ],
            axis=1,
        )
        in_maps.append(
            {
                "xT": xT[b],
                "wqkv": np.ascontiguousarray(wqkv).astype(bf16),
                "oproj": np.ascontiguousarray(
                    o_proj[:, QCOLS * g : QCOLS * g + QCOLS].T
                ).astype(bf16),
                "masks": masks,
                "ident": ident,
                "ones": np.ones((128, 16 * 65), dtype=bf16),
            }
        )
    return in_maps


def run(x, q_proj, k_proj, v_proj, o_proj, trace=False):
    """Run on hardware; returns (output [B,T,D] f32, BassKernelResults)."""
    if "nc" not in _cache:
        _cache["nc"] = _build()
    nc = _cache["nc"]
    in_maps = _host_inputs(x, q_proj, k_proj, v_proj, o_proj)
    res = run_bass_kernel_spmd(
        nc, in_maps, core_ids=list(range(N_CORES)), trace=trace
    )
    parts = [res.results[c]["out"] for c in range(N_CORES)]
    out = np.empty((B, T, D), dtype=np.float32)
    for b in range(B):
        acc = parts[4 * b].astype(np.float64)
        for g in range(1, GROUPS):
            acc += parts[4 * b + g]
        out[b] = acc.astype(np.float32)
    return out, res


def kernel(x, q_proj, k_proj, v_proj, o_proj, hq=None, hk=None, **_unused):
    x = np.asarray(x, dtype=np.float32)
    q_proj = np.asarray(q_proj, dtype=np.float32)
    k_proj = np.asarray(k_proj, dtype=np.float32)
    v_proj = np.asarray(v_proj, dtype=np.float32)
    o_proj = np.asarray(o_proj, dtype=np.float32)
    assert x.shape == (B, T, D), x.shape
    trace = bool(os.environ.get("KERNEL_TRACE"))
    out, _ = run(x, q_proj, k_proj, v_proj, o_proj, trace=trace)
    return out


# revision 7
# speedup vs baseline: 1.4388x; 1.0749x over previous
"""GQA self-attention block (q/k/v proj + causal softmax attention + o proj)
on 8 trn2 NeuronCores.

Sharding: batch (2) x query-head-groups (4) -> 8 cores. Core c handles
batch b=c//4 and heads [8g, 8g+8) where g=c%4 (kv heads [2g, 2g+2)).
Each core computes a partial output [T, D] = ctx_heads @ o_proj_cols.T;
the host sums the 4 partials per batch (all-reduce done host-side).

All matmuls bf16 (FWL weight loads + 1 col/cycle streams), fp32 PSUM.

Single software-pipelined schedule: query blocks a=0..3 outer, head pairs
m inner. Phase-1 token-block b=a+1 and phase-3 output tiles of block a-1
are interleaved into the attention chunk stream as PE filler so the tensor
engine never idles while ACT works through the exps. DMAs are merged into
few large transfers (descriptors spread across all 16 SDMA engines) to
keep the Sync descriptor-generation queue off the critical path, and the
softmax normalizer broadcast runs as a K=1 ones-matmul on the PE instead
of a DRAM round-trip.
"""

import os
import sys

sys.path.insert(0, "/opt/trn_rl_repo")

import numpy as np

import concourse.bass as bass
import concourse.tile as tile
from concourse import bacc, mybir
from concourse.bass_utils import run_bass_kernel_spmd

F32 = mybir.dt.float32
BF16 = mybir.dt.bfloat16
EXP = mybir.ActivationFunctionType.Exp

B, T, D = 2, 2048, 2048
HQ, HK = 32, 8
N_CORES = 8
GROUPS = 4
QCOLS = D // GROUPS         # 512 q cols per core
KCOLS = (D // 4) // GROUPS  # 128 k cols per core (2 kv heads)
WCOLS = QCOLS + 2 * KCOLS   # 768
TB = 512                    # phase-1 token block
NTB = T // TB               # 4
KT = D // 128               # 16 contraction tiles
NEG = -480.0                # additive mask pre-scale (-60 after 1/8 scale)

_cache = {}


def _build():
    nc = bacc.Bacc("TRN2", target_bir_lowering=False, debug=False)

    xT_d = nc.declare_dram_parameter("xT", [D, T], BF16, isOutput=False)
    wqkv_d = nc.declare_dram_parameter("wqkv", [D, WCOLS], BF16, isOutput=False)
    oproj_d = nc.declare_dram_parameter("oproj", [QCOLS, D], BF16, isOutput=False)
    masks_d = nc.declare_dram_parameter("masks", [128, 2, 128], BF16, isOutput=False)
    ident_d = nc.declare_dram_parameter("ident", [128, 64], BF16, isOutput=False)
    ones_d = nc.declare_dram_parameter("ones", [128, 16 * 65], BF16, isOutput=False)
    out_d = nc.declare_dram_parameter("out", [T, D], F32, isOutput=True)

    xT_r = xT_d[:].rearrange("(k p) t -> p k t", p=128)

    with tile.TileContext(nc) as tc:
        with (
            tc.tile_pool(name="pers", bufs=1) as pers,
            tc.tile_pool(name="xt", bufs=2) as xtp,
            tc.tile_pool(name="work", bufs=2) as work,
            tc.tile_pool(name="psum", bufs=1, space="PSUM") as psum,
        ):
            # ---- persistent tiles ----
            wqkv_sb = pers.tile([128, KT, WCOLS], BF16, tag="wslot")
            oproj_sb = pers.tile([128, 4, D], BF16, tag="oslot")
            masks_sb = pers.tile([128, 2, 128], BF16, tag="masks")
            ident_sb = pers.tile([128, 64], BF16, tag="ident")
            ones64 = pers.tile([1, 64], BF16, tag="ones64")
            qt = [pers.tile([128, T], BF16, tag=f"qt{m}", name=f"qt{m}")
                  for m in range(4)]
            kp = [pers.tile([128, T], BF16, tag=f"kp{k}", name=f"kp{k}")
                  for k in range(2)]
            vT = pers.tile([128, T], BF16, tag="vT")
            vs = [pers.tile([128, 16 * 65], BF16, tag=f"vs{k}", name=f"vs{k}")
                  for k in range(2)]
            ctx = [pers.tile([128, T], BF16, tag=f"ctx{m}", name=f"ctx{m}")
                   for m in range(4)]

            # ---- input DMAs for tb0 + constants ----
            # tb0's x arrives in 4 k-chunks so the first matmul group can
            # start before the whole block lands
            wq_r = wqkv_d[:].rearrange("(k p) c -> p k c", p=128)
            xt0 = xtp.tile([128, KT, TB], BF16, tag="xt", bufs=2, name="xt0")
            for kc in range(4):
                nc.sync.dma_start(
                    xt0[:, 4 * kc : 4 * kc + 4, :],
                    xT_r[:, 4 * kc : 4 * kc + 4, 0:TB],
                )
            nc.sync.dma_start(wqkv_sb[:, :, 0:128], wq_r[:, :, 0:128])
            nc.sync.dma_start(wqkv_sb[:, :, 128:WCOLS], wq_r[:, :, 128:WCOLS])
            nc.sync.dma_start(masks_sb, masks_d[:])
            nc.sync.dma_start(ident_sb, ident_d[:])
            nc.sync.dma_start(ones64, ones_d[0:1, 0:64])
            nc.sync.dma_start(vs[0], ones_d[:])
            nc.sync.dma_start(vs[1], ones_d[:])
            nc.sync.dma_start(
                oproj_sb, oproj_d[:].rearrange("(m p) c -> p m c", p=128)
            )

            # ---- phase-1 emission helpers ----
            def p1_dma(tb):
                xts = xtp.tile([128, KT, TB], BF16, tag="xt", bufs=2, name="xt")
                ts = slice(tb * TB, tb * TB + TB)
                nc.sync.dma_start(xts, xT_r[:, :, ts])
                return xts

            def p1_group(tb, m, xts):
                ts = slice(tb * TB, tb * TB + TB)
                ps = psum.tile([128, TB], F32, tag="s2", bufs=4)
                for k in range(KT):
                    nc.tensor.matmul(
                        ps,
                        wqkv_sb[:, k, 128 * m : 128 * m + 128],
                        xts[:, k, :],
                        start=(k == 0),
                        stop=(k == KT - 1),
                    )
                if m < 4:
                    nc.vector.tensor_copy(qt[m][:, ts], ps)
                elif m == 4:
                    nc.vector.tensor_copy(kp[0][0:64, ts], ps[0:64, :])
                    nc.vector.tensor_copy(kp[1][64:128, ts], ps[64:128, :])
                else:
                    nc.vector.tensor_copy(vT[:, ts], ps)

            def p1_final(tb):
                # k duplicate at the other partition half + v transposes
                ts = slice(tb * TB, tb * TB + TB)
                nc.sync.dma_start(kp[0][64:128, ts], kp[0][0:64, ts])
                nc.sync.dma_start(kp[1][0:64, ts], kp[1][64:128, ts])
                for kv in range(2):
                    rows = slice(64 * kv, 64 * kv + 64)
                    for ci in range(4):
                        c = 4 * tb + ci
                        tp = psum.tile([128, 64], BF16, tag="s2", bufs=4)
                        nc.tensor.transpose(
                            tp,
                            vT[rows, 128 * c : 128 * c + 128],
                            ident_sb[rows, :],
                            tile_position=(64 * kv, 0),
                        )
                        nc.vector.tensor_copy(vs[kv][:, 65 * c : 65 * c + 64], tp)

            # ---- phase-3 emission helper (one [128-token, 1024-col] unit) ----
            def p3_unit(t, rp):
                tsl = slice(128 * t, 128 * t + 128)
                ps = psum.tile([128, 1024], F32, tag="s2", bufs=4)
                for m in range(4):
                    for half in range(2):
                        r = 2 * rp + half
                        nc.tensor.matmul(
                            ps[:, 512 * half : 512 * half + 512],
                            ctx[m][:, tsl],
                            oproj_sb[:, m, 512 * r : 512 * r + 512],
                            start=(m == 0),
                            stop=(m == 3),
                        )
                ostage = work.tile([128, 1024], F32, tag="ost", bufs=4,
                                   name="ostage")
                if rp == 0:
                    nc.vector.tensor_copy(ostage, ps)
                else:
                    nc.scalar.copy(ostage, ps)
                nc.sync.dma_start(
                    out_d[tsl, 1024 * rp : 1024 * rp + 1024], ostage
                )

            # ---- run phase 1 for tb0 ----
            for m in range(6):
                p1_group(0, m, xt0)
            p1_final(0)

            # ---- main pipeline: a outer, m inner ----
            pending_norm = []

            def emit_norm(cu, m, a):
                isl = slice(512 * a, 512 * a + 512)

                def _normalize(cu=cu, m=m, a=a, isl=isl):
                    # denominator row folded to [128,8] so the reciprocal runs
                    # on all lanes, then unfolded to a [1,1024] bf16 row and
                    # broadcast across 64 partitions with a K=1 ones-matmul
                    den128 = work.tile([128, 8], F32, tag="d128", bufs=2,
                                       name="den128")
                    nc.sync.dma_start(den128, cu[64:65, :])
                    rcf = work.tile([128, 8], F32, tag="rcf", bufs=2,
                                    name="rcf")
                    nc.vector.reciprocal_approx_fast(rcf, den128)
                    rcb = work.tile([128, 8], BF16, tag="rcb", bufs=2,
                                    name="rcb")
                    nc.vector.tensor_copy(rcb, rcf)
                    rcp = work.tile([1, 1024], BF16, tag="rcp", bufs=2,
                                    name="rcp")
                    nc.sync.dma_start(rcp, rcb)
                    bcs = psum.tile([64, 1024], F32, tag="s2", bufs=4)
                    for h in range(2):
                        nc.tensor.matmul(
                            bcs[:, 512 * h : 512 * h + 512],
                            ones64,
                            rcp[:, 512 * h : 512 * h + 512],
                            start=True,
                            stop=True,
                        )
                    nc.vector.tensor_mul(
                        ctx[m][0:64, isl], cu[0:64, 0:512], bcs[:, 0:512]
                    )
                    tmpB = work.tile([64, 512], BF16, tag="tb", bufs=2)
                    nc.vector.tensor_mul(
                        tmpB, cu[0:64, 512:1024], bcs[:, 512:1024]
                    )
                    nc.sync.dma_start(ctx[m][64:128, isl], tmpB)

                pending_norm.append(_normalize)

            for a in range(4):
                nj = 4 * (a + 1)

                # plan PE filler work for this a-block, keyed by head pair m
                fillers = {0: [], 1: [], 2: [], 3: []}
                if a + 1 < NTB:
                    tbn = a + 1
                    xts_n = p1_dma(tbn)
                    for mm in range(6):
                        fillers[mm // 3].append(
                            lambda tb=tbn, mm=mm, xx=xts_n: p1_group(tb, mm, xx)
                        )
                    fillers[1].append(lambda tb=tbn: p1_final(tb))
                if a >= 1:
                    ap = a - 1
                    for ui, (t, rp) in enumerate(
                        (4 * ap + ti, rp) for ti in range(4) for rp in range(2)
                    ):
                        fillers[2 + (ui // 4)].append(
                            lambda t=t, rp=rp: p3_unit(t, rp)
                        )

                for m in range(4):
                    kv = m // 2
                    ctxAB = psum.tile([65, 1024], F32, tag="s2", bufs=4)
                    pend = []
                    fl = fillers[m]
                    # spread fillers across the chunk loop (from jc=3 so they
                    # don't collide with the jc==2 norm flush)
                    fpos = {}
                    if fl:
                        step = max(1, (nj - 3) // len(fl)) if nj > 3 else 1
                        for fi in range(len(fl)):
                            fpos.setdefault(min(nj - 1, fi * step + 3), []).append(fl[fi])
                    norm_keep = 2 if a < 3 else 1
                    for jc in range(nj):
                        if jc == 2 and len(pending_norm) >= norm_keep:
                            pending_norm.pop(0)()
                        o = jc - 4 * a
                        lo = 128 * o if o >= 0 else 0
                        jsl = slice(128 * jc, 128 * jc + 128)
                        S = psum.tile([128, 2, 512], F32, tag="s2", bufs=4)
                        for h2 in range(2):
                            nc.tensor.matmul(
                                S[:, h2, lo:512],
                                kp[kv][64 * h2 : 64 * h2 + 64, jsl],
                                qt[m][64 * h2 : 64 * h2 + 64,
                                      512 * a + lo : 512 * a + 512],
                                start=True,
                                stop=True,
                                tile_position=(64 * h2, 0),
                            )
                        if o >= 0:
                            # triangle mask on the diagonal 128-col block,
                            # both heads in one 2-range DVE add
                            nc.vector.tensor_add(
                                S[:, :, lo : lo + 128],
                                S[:, :, lo : lo + 128],
                                masks_sb[:, :, :],
                            )
                        E = work.tile([128, 2, 512], BF16, tag="E", bufs=4)
                        nc.scalar.activation(
                            E[:, :, lo:512], S[:, :, lo:512], EXP, scale=0.125
                        )
                        pend.append((E, jc, lo))
                        if len(pend) > 2:
                            pE, pjc, plo = pend.pop(0)
                            for h2 in range(2):
                                nc.tensor.matmul(
                                    ctxAB[:, 512 * h2 + plo : 512 * h2 + 512],
                                    vs[kv][:, 65 * pjc : 65 * pjc + 65],
                                    pE[:, h2, plo:512],
                                    start=(pjc == 0),
                                    stop=False,
                                )
                        for f in fpos.get(jc, ()):
                            f()
                    while pend:
                        pE, pjc, plo = pend.pop(0)
                        for h2 in range(2):
                            nc.tensor.matmul(
                                ctxAB[:, 512 * h2 + plo : 512 * h2 + 512],
                                vs[kv][:, 65 * pjc : 65 * pjc + 65],
                                pE[:, h2, plo:512],
                                start=(pjc == 0),
                                stop=(pjc == nj - 1),
                            )

                    # fast PSUM evacuation; the normalize chain runs deferred
                    cu = work.tile([65, 1024], F32, tag="cu", bufs=3, name="cu")
                    nc.vector.tensor_copy(cu, ctxAB)
                    emit_norm(cu, m, a)

            # ---- drain: last norms + last phase-3 block ----
            while pending_norm:
                pending_norm.pop(0)()
            for ti in range(4):
                for rp in range(2):
                    p3_unit(12 + ti, rp)

    nc.compile()
    return nc


def _host_inputs(x, q_proj, k_proj, v_proj, o_proj):
    """Per-core input dicts (numpy, bf16 on-device dtypes)."""
    import ml_dtypes

    bf16 = ml_dtypes.bfloat16
    masks = np.zeros((128, 2, 128), dtype=np.float32)
    jj = np.arange(128)[:, None]
    ii = np.arange(128)[None, :]
    tri = np.where(jj <= ii, 0.0, NEG)
    masks[:, 0, :] = tri
    masks[:, 1, :] = tri
    masks = masks.astype(bf16)
    ident = np.zeros((128, 64), dtype=np.float32)
    ident[np.arange(64), np.arange(64)] = 1.0
    ident[np.arange(64) + 64, np.arange(64)] = 1.0
    ident = ident.astype(bf16)

    xT = [np.ascontiguousarray(x[b].T).astype(bf16) for b in range(B)]
    in_maps = []
    for c in range(N_CORES):
        b, g = divmod(c, GROUPS)
        wqkv = np.concatenate(
            [
                q_proj[QCOLS * g : QCOLS * g + QCOLS].T,
                k_proj[KCOLS * g : KCOLS * g + KCOLS].T,
                v_proj[KCOLS * g : KCOLS * g + KCOLS].T,
            ],
            axis=1,
        )
        in_maps.append(
            {
                "xT": xT[b],
                "wqkv": np.ascontiguousarray(wqkv).astype(bf16),
                "oproj": np.ascontiguousarray(
                    o_proj[:, QCOLS * g : QCOLS * g + QCOLS].T
                ).astype(bf16),
                "masks": masks,
                "ident": ident,
                "ones": np.ones((128, 16 * 65), dtype=bf16),
            }
        )
    return in_maps


def run(x, q_proj, k_proj, v_proj, o_proj, trace=False):
    """Run on hardware; returns (output [B,T,D] f32, BassKernelResults)."""
    if "nc" not in _cache:
        _cache["nc"] = _build()
    nc = _cache["nc"]
    in_maps = _host_inputs(x, q_proj, k_proj, v_proj, o_proj)
    res = run_bass_kernel_spmd(
        nc, in_maps, core_ids=list(range(N_CORES)), trace=trace
    )
    parts = [res.results[c]["out"] for c in range(N_CORES)]
    out = np.empty((B, T, D), dtype=np.float32)
    for b in range(B):
        acc = parts[4 * b].astype(np.float64)
        for g in range(1, GROUPS):
            acc += parts[4 * b + g]
        out[b] = acc.astype(np.float32)
    return out, res


def kernel(x, q_proj, k_proj, v_proj, o_proj, hq=None, hk=None, **_unused):
    x = np.asarray(x, dtype=np.float32)
    q_proj = np.asarray(q_proj, dtype=np.float32)
    k_proj = np.asarray(k_proj, dtype=np.float32)
    v_proj = np.asarray(v_proj, dtype=np.float32)
    o_proj = np.asarray(o_proj, dtype=np.float32)
    assert x.shape == (B, T, D), x.shape
    trace = bool(os.environ.get("KERNEL_TRACE"))
    out, _ = run(x, q_proj, k_proj, v_proj, o_proj, trace=trace)
    return out


# revision 12
# speedup vs baseline: 1.4795x; 1.0283x over previous
"""GQA self-attention block (q/k/v proj + causal softmax attention + o proj)
on 8 trn2 NeuronCores.

Sharding: batch (2) x query-head-groups (4) -> 8 cores. Core c handles
batch b=c//4 and heads [8g, 8g+8) where g=c%4 (kv heads [2g, 2g+2)).
Each core computes a partial output [T, D] = ctx_heads @ o_proj_cols.T;
the host sums the 4 partials per batch (all-reduce done host-side).

All matmuls bf16 (FWL weight loads + 1 col/cycle streams), fp32 PSUM.

Single software-pipelined schedule: query blocks a=0..3 outer, head pairs
m inner. Phase-1 token-block b=a+1 and phase-3 output tiles of block a-1
are interleaved into the attention chunk stream as PE filler so the tensor
engine never idles while ACT works through the exps. DMAs are merged into
few large transfers (descriptors spread across all 16 SDMA engines) to
keep the Sync descriptor-generation queue off the critical path, and the
softmax normalizer broadcast runs as a K=1 ones-matmul on the PE instead
of a DRAM round-trip.
"""

import os
import sys

sys.path.insert(0, "/opt/trn_rl_repo")

import numpy as np

import concourse.bass as bass
import concourse.tile as tile
from concourse import bacc, mybir
from concourse.bass_utils import run_bass_kernel_spmd

F32 = mybir.dt.float32
BF16 = mybir.dt.bfloat16
EXP = mybir.ActivationFunctionType.Exp

B, T, D = 2, 2048, 2048
HQ, HK = 32, 8
N_CORES = 8
GROUPS = 4
QCOLS = D // GROUPS         # 512 q cols per core
KCOLS = (D // 4) // GROUPS  # 128 k cols per core (2 kv heads)
WCOLS = QCOLS + 2 * KCOLS   # 768
TB = 512                    # phase-1 token block
NTB = T // TB               # 4
KT = D // 128               # 16 contraction tiles
NEG = -480.0                # additive mask pre-scale (-60 after 1/8 scale)

_cache = {}


def _build():
    nc = bacc.Bacc("TRN2", target_bir_lowering=False, debug=False)

    xT_d = nc.declare_dram_parameter("xT", [D, T], BF16, isOutput=False)
    # wqkv comes pre-blocked per 128-col output group so each group is one
    # contiguous DMA
    wqkv_d = nc.declare_dram_parameter("wqkv", [6, 128, KT, 128], BF16,
                                       isOutput=False)
    oproj_d = nc.declare_dram_parameter("oproj", [QCOLS, D], BF16, isOutput=False)
    masks_d = nc.declare_dram_parameter("masks", [128, 2, 128], BF16, isOutput=False)
    ident_d = nc.declare_dram_parameter("ident", [128, 64], BF16, isOutput=False)
    ones_d = nc.declare_dram_parameter("ones", [128, 16 * 65], BF16, isOutput=False)
    out_d = nc.declare_dram_parameter("out", [T, D], F32, isOutput=True)

    xT_r = xT_d[:].rearrange("(k p) t -> p k t", p=128)

    with tile.TileContext(nc) as tc:
        with (
            tc.tile_pool(name="pers", bufs=1) as pers,
            tc.tile_pool(name="xt", bufs=2) as xtp,
            tc.tile_pool(name="work", bufs=2) as work,
            tc.tile_pool(name="psum", bufs=1, space="PSUM") as psum,
        ):
            # ---- persistent tiles ----
            wqkv_sb = pers.tile([128, KT, WCOLS], BF16, tag="wslot")
            oproj_sb = pers.tile([128, 4, D], BF16, tag="oslot")
            masks_sb = pers.tile([128, 2, 128], BF16, tag="masks")
            ident_sb = pers.tile([128, 64], BF16, tag="ident")
            ones64 = pers.tile([1, 64], BF16, tag="ones64")
            qt = [pers.tile([128, T], BF16, tag=f"qt{m}", name=f"qt{m}")
                  for m in range(4)]
            kp = [pers.tile([128, T], BF16, tag=f"kp{k}", name=f"kp{k}")
                  for k in range(2)]
            vT = pers.tile([128, T], BF16, tag="vT")
            vs = [pers.tile([128, 16 * 65], BF16, tag=f"vs{k}", name=f"vs{k}")
                  for k in range(2)]
            ctx = [pers.tile([128, T], BF16, tag=f"ctx{m}", name=f"ctx{m}")
                   for m in range(4)]

            # ---- input DMAs for tb0 + constants ----
            # tb0's x arrives in 4 k-chunks so the first matmul group can
            # start before the whole block lands; wqkv per 128-col group
            xt0 = xtp.tile([128, KT, TB], BF16, tag="xt", bufs=2, name="xt0")
            nc.sync.dma_start(wqkv_sb[:, :, 0:128], wqkv_d[0])
            for kc in range(4):
                nc.sync.dma_start(
                    xt0[:, 4 * kc : 4 * kc + 4, :],
                    xT_r[:, 4 * kc : 4 * kc + 4, 0:TB],
                )
            for g in range(1, 6):
                nc.sync.dma_start(
                    wqkv_sb[:, :, 128 * g : 128 * g + 128], wqkv_d[g]
                )
            nc.sync.dma_start(masks_sb, masks_d[:])
            nc.sync.dma_start(ident_sb, ident_d[:])
            nc.sync.dma_start(ones64, ones_d[0:1, 0:64])
            nc.sync.dma_start(vs[0], ones_d[:])
            nc.sync.dma_start(vs[1], ones_d[:])
            nc.sync.dma_start(
                oproj_sb, oproj_d[:].rearrange("(m p) c -> p m c", p=128)
            )

            # ---- phase-1 emission helpers ----
            def p1_dma(tb):
                xts = xtp.tile([128, KT, TB], BF16, tag="xt", bufs=2, name="xt")
                ts = slice(tb * TB, tb * TB + TB)
                nc.sync.dma_start(xts, xT_r[:, :, ts])
                return xts

            def p1_group(tb, m, xts):
                ts = slice(tb * TB, tb * TB + TB)
                ps = psum.tile([128, TB], F32, tag="s2", bufs=4)
                for k in range(KT):
                    nc.tensor.matmul(
                        ps,
                        wqkv_sb[:, k, 128 * m : 128 * m + 128],
                        xts[:, k, :],
                        start=(k == 0),
                        stop=(k == KT - 1),
                    )
                if m < 4:
                    nc.vector.tensor_copy(qt[m][:, ts], ps)
                elif m == 4:
                    nc.vector.tensor_copy(kp[0][0:64, ts], ps[0:64, :])
                    nc.vector.tensor_copy(kp[1][64:128, ts], ps[64:128, :])
                else:
                    nc.vector.tensor_copy(vT[:, ts], ps)

            def p1_final(tb):
                # k duplicate at the other partition half + v transposes
                ts = slice(tb * TB, tb * TB + TB)
                nc.sync.dma_start(kp[0][64:128, ts], kp[0][0:64, ts])
                nc.sync.dma_start(kp[1][0:64, ts], kp[1][64:128, ts])
                for kv in range(2):
                    rows = slice(64 * kv, 64 * kv + 64)
                    for ci in range(4):
                        c = 4 * tb + ci
                        tp = psum.tile([128, 64], BF16, tag="s2", bufs=4)
                        nc.tensor.transpose(
                            tp,
                            vT[rows, 128 * c : 128 * c + 128],
                            ident_sb[rows, :],
                            tile_position=(64 * kv, 0),
                        )
                        nc.vector.tensor_copy(vs[kv][:, 65 * c : 65 * c + 64], tp)

            # ---- phase-3 emission helper (one [128-token, 1024-col] unit) ----
            def p3_unit(t, rp):
                tsl = slice(128 * t, 128 * t + 128)
                ps = psum.tile([128, 1024], F32, tag="s2", bufs=4)
                for m in range(4):
                    for half in range(2):
                        r = 2 * rp + half
                        nc.tensor.matmul(
                            ps[:, 512 * half : 512 * half + 512],
                            ctx[m][:, tsl],
                            oproj_sb[:, m, 512 * r : 512 * r + 512],
                            start=(m == 0),
                            stop=(m == 3),
                        )
                ostage = work.tile([128, 1024], F32, tag="ost", bufs=4,
                                   name="ostage")
                if rp == 0:
                    nc.vector.tensor_copy(ostage, ps)
                else:
                    nc.scalar.copy(ostage, ps)
                nc.sync.dma_start(
                    out_d[tsl, 1024 * rp : 1024 * rp + 1024], ostage
                )

            # ---- run phase 1 for tb0 ----
            for m in range(6):
                p1_group(0, m, xt0)
            p1_final(0)

            # ---- main pipeline: a outer, m inner ----
            pending_norm = []

            def emit_norm(cu, m, a):
                isl = slice(512 * a, 512 * a + 512)

                def _normalize(cu=cu, m=m, a=a, isl=isl):
                    # denominator row folded to [128,8] so the reciprocal runs
                    # on all lanes, then unfolded to a [1,1024] bf16 row and
                    # broadcast across 64 partitions with a K=1 ones-matmul
                    den128 = work.tile([128, 8], F32, tag="d128", bufs=2,
                                       name="den128")
                    nc.sync.dma_start(den128, cu[64:65, :])
                    rcf = work.tile([128, 8], F32, tag="rcf", bufs=2,
                                    name="rcf")
                    nc.vector.reciprocal_approx_fast(rcf, den128)
                    rcb = work.tile([128, 8], BF16, tag="rcb", bufs=2,
                                    name="rcb")
                    nc.vector.tensor_copy(rcb, rcf)
                    rcp = work.tile([1, 1024], BF16, tag="rcp", bufs=2,
                                    name="rcp")
                    nc.sync.dma_start(rcp, rcb)
                    bcs = psum.tile([64, 1024], F32, tag="s2", bufs=4)
                    for h in range(2):
                        nc.tensor.matmul(
                            bcs[:, 512 * h : 512 * h + 512],
                            ones64,
                            rcp[:, 512 * h : 512 * h + 512],
                            start=True,
                            stop=True,
                        )
                    nc.vector.tensor_mul(
                        ctx[m][0:64, isl], cu[0:64, 0:512], bcs[:, 0:512]
                    )
                    tmpB = work.tile([64, 512], BF16, tag="tb", bufs=2)
                    nc.vector.tensor_mul(
                        tmpB, cu[0:64, 512:1024], bcs[:, 512:1024]
                    )
                    nc.sync.dma_start(ctx[m][64:128, isl], tmpB)

                pending_norm.append(_normalize)

            # ---- flat chunk schedule across all (a, m) blocks so the
            # pipeline never drains at block boundaries ----
            chunks = []
            block_base = {}
            for a in range(4):
                nj = 4 * (a + 1)
                for m in range(4):
                    block_base[(a, m)] = len(chunks)
                    for jc in range(nj):
                        chunks.append((a, m, m // 2, jc, nj))

            # filler closures at global chunk positions
            filler_at = {}

            def add_filler(pos, f):
                filler_at.setdefault(min(pos, len(chunks) - 1), []).append(f)

            for a in range(4):
                nj = 4 * (a + 1)
                per_m = {0: [], 1: [], 2: [], 3: []}
                if a + 1 < NTB:
                    tbn = a + 1
                    for mm in range(6):
                        per_m[mm // 3].append(
                            lambda tb=tbn, mm=mm: p1_group(tb, mm, xts_of[tb])
                        )
                    per_m[1].append(lambda tb=tbn: p1_final(tb))
                if a >= 1:
                    ap = a - 1
                    for ui, (t, rp) in enumerate(
                        (4 * ap + ti, rp) for ti in range(4) for rp in range(2)
                    ):
                        per_m[2 + (ui // 4)].append(
                            lambda t=t, rp=rp: p3_unit(t, rp)
                        )
                for m in range(4):
                    fl = per_m[m]
                    if not fl:
                        continue
                    step = max(1, (nj - 3) // len(fl)) if nj > 3 else 1
                    for fi in range(len(fl)):
                        add_filler(block_base[(a, m)] + 3 + fi * step, fl[fi])

            xts_of = {}
            ctxAB_of = {}

            def issue_ctx(rec):
                pE, a, m, kv, jc, lo, nj = rec
                key = (a, m)
                if key not in ctxAB_of:
                    ctxAB_of[key] = psum.tile([65, 1024], F32, tag="s2",
                                              bufs=4, name="ctxAB")
                ctxAB = ctxAB_of[key]
                for h2 in range(2):
                    nc.tensor.matmul(
                        ctxAB[:, 512 * h2 + lo : 512 * h2 + 512],
                        vs[kv][:, 65 * jc : 65 * jc + 65],
                        pE[:, h2, lo:512],
                        start=(jc == 0),
                        stop=(jc == nj - 1),
                    )
                if jc == nj - 1:
                    # fast PSUM evacuation; normalize runs deferred
                    cu = work.tile([65, 1024], F32, tag="cu", bufs=3,
                                   name="cu")
                    nc.vector.tensor_copy(cu, ctxAB)
                    emit_norm(cu, m, a)
                    del ctxAB_of[key]

            pend = []
            for ci, (a, m, kv, jc, nj) in enumerate(chunks):
                if jc == 0 and m == 0 and a + 1 < NTB:
                    xts_of[a + 1] = p1_dma(a + 1)
                if jc == 2 and len(pending_norm) >= (2 if a < 3 else 1):
                    pending_norm.pop(0)()
                o = jc - 4 * a
                lo = 128 * o if o >= 0 else 0
                jsl = slice(128 * jc, 128 * jc + 128)
                S = psum.tile([128, 2, 512], F32, tag="s2", bufs=4)
                for h2 in range(2):
                    nc.tensor.matmul(
                        S[:, h2, lo:512],
                        kp[kv][64 * h2 : 64 * h2 + 64, jsl],
                        qt[m][64 * h2 : 64 * h2 + 64,
                              512 * a + lo : 512 * a + 512],
                        start=True,
                        stop=True,
                        tile_position=(64 * h2, 0),
                    )
                if o >= 0:
                    # triangle mask on the diagonal 128-col block, both
                    # heads in one 2-range DVE add
                    nc.vector.tensor_add(
                        S[:, :, lo : lo + 128],
                        S[:, :, lo : lo + 128],
                        masks_sb[:, :, :],
                    )
                E = work.tile([128, 2, 512], BF16, tag="E", bufs=4)
                nc.scalar.activation(
                    E[:, :, lo:512], S[:, :, lo:512], EXP, scale=0.125
                )
                pend.append((E, a, m, kv, jc, lo, nj))
                if len(pend) > 2:
                    issue_ctx(pend.pop(0))
                for f in filler_at.get(ci, ()):
                    f()
            while pend:
                issue_ctx(pend.pop(0))

            # ---- drain: last norms + last phase-3 block ----
            while pending_norm:
                pending_norm.pop(0)()
            for ti in range(4):
                for rp in range(2):
                    p3_unit(12 + ti, rp)

    nc.compile()
    return nc


def _host_inputs(x, q_proj, k_proj, v_proj, o_proj):
    """Per-core input dicts (numpy, bf16 on-device dtypes)."""
    import ml_dtypes

    bf16 = ml_dtypes.bfloat16
    masks = np.zeros((128, 2, 128), dtype=np.float32)
    jj = np.arange(128)[:, None]
    ii = np.arange(128)[None, :]
    tri = np.where(jj <= ii, 0.0, NEG)
    masks[:, 0, :] = tri
    masks[:, 1, :] = tri
    masks = masks.astype(bf16)
    ident = np.zeros((128, 64), dtype=np.float32)
    ident[np.arange(64), np.arange(64)] = 1.0
    ident[np.arange(64) + 64, np.arange(64)] = 1.0
    ident = ident.astype(bf16)

    xT = [np.ascontiguousarray(x[b].T).astype(bf16) for b in range(B)]
    in_maps = []
    for c in range(N_CORES):
        b, g = divmod(c, GROUPS)
        wqkv = np.concatenate(
            [
                q_proj[QCOLS * g : QCOLS * g + QCOLS].T,
                k_proj[KCOLS * g : KCOLS * g + KCOLS].T,
                v_proj[KCOLS * g : KCOLS * g + KCOLS].T,
            ],
            axis=1,
        )
        # pre-block [D, WCOLS] -> [group, partition, ktile, col] so each
        # 128-col output group is one contiguous DMA
        wqkv_blk = np.ascontiguousarray(
            wqkv.reshape(KT, 128, 6, 128).transpose(2, 1, 0, 3)
        )
        in_maps.append(
            {
                "xT": xT[b],
                "wqkv": wqkv_blk.astype(bf16),
                "oproj": np.ascontiguousarray(
                    o_proj[:, QCOLS * g : QCOLS * g + QCOLS].T
                ).astype(bf16),
                "masks": masks,
                "ident": ident,
                "ones": np.ones((128, 16 * 65), dtype=bf16),
            }
        )
    return in_maps


def run(x, q_proj, k_proj, v_proj, o_proj, trace=False):
    """Run on hardware; returns (output [B,T,D] f32, BassKernelResults)."""
    if "nc" not in _cache:
        _cache["nc"] = _build()
    nc = _cache["nc"]
    in_maps = _host_inputs(x, q_proj, k_proj, v_proj, o_proj)
    res = run_bass_kernel_spmd(
        nc, in_maps, core_ids=list(range(N_CORES)), trace=trace
    )
    parts = [res.results[c]["out"] for c in range(N_CORES)]
    out = np.empty((B, T, D), dtype=np.float32)
    for b in range(B):
        acc = parts[4 * b].astype(np.float64)
        for g in range(1, GROUPS):
            acc += parts[4 * b + g]
        out[b] = acc.astype(np.float32)
    return out, res


def kernel(x, q_proj, k_proj, v_proj, o_proj, hq=None, hk=None, **_unused):
    x = np.asarray(x, dtype=np.float32)
    q_proj = np.asarray(q_proj, dtype=np.float32)
    k_proj = np.asarray(k_proj, dtype=np.float32)
    v_proj = np.asarray(v_proj, dtype=np.float32)
    o_proj = np.asarray(o_proj, dtype=np.float32)
    assert x.shape == (B, T, D), x.shape
    trace = bool(os.environ.get("KERNEL_TRACE"))
    out, _ = run(x, q_proj, k_proj, v_proj, o_proj, trace=trace)
    return out
